# revision 45
# baseline (speedup 1.0000x reference)
"""Trainium2 Bass kernel for GQA MultiHeadAttention + LoRA + ALiBi + causal mask.

Problem (hardcoded): B=2, S=2048, D=1024, H=16 q-heads, KVH=4 kv-heads, DK=64,
LoRA rank 16, ALiBi alpha 1.0, causal, softmax, eval mode.

Sharding over 8 cores: core c = 4*b + g handles batch b and kv-group g
(query heads 4g..4g+3, kv head g).

Host precompute:
  - LoRA folded into weights:  W_eff = W + A@B (SCALING=1).
  - Score scale (1/sqrt(64)) folded into W_eff_q.
  - Inputs passed pre-transposed (xT = x.T) because the PE contracts over the
    partition dim.
  - ALiBi folded into the score matmul as 2 extra contraction rows:
      kext = [k_idx; 1],  qext_h = [slope_h; -slope_h * q_idx]
    so S'[k,q] = scale*q.k + slope_h*(k - q) comes out of one matmul (K=66).

Device dataflow (per core, all matmul operands float32r = full-rate fp32):
  1. Projections into transposed layouts: Q^T (per head, 66 rows with ext),
     K^T (66 rows with ext), V natural [k,64] + ones column (row sums ride
     along the attention matmul: O^T_ext row 64 = softmax denominator).
  2. Per q-block of 512, per k-tile of 128: S^T = K^T_ext.T @ Q^T_ext on PE
     (contraction 66, [k, q] layout), exp on ACT over 2-head groups,
     causal diagonal handled by narrowing the q-span + one additive
     [128,128] mask (0 / -1e30) before exp, O^T accumulated in PSUM.
  3. Normalize: reciprocal of denominator row (DVE), broadcast across
     partitions (GpSimd), multiply (DVE).
  4. Output projection computed transposed (Y^T), its 8 column-tile units
     drip-fed into the next q-block's PE stream; host sums the 4 partial
     Y^T per batch and transposes.

  Schedule: proj(sb) / attention(sb-1) / outproj(sb-1) software-pipelined;
  PSUM budget 2 (proj+outproj) + 4 (scores, double-buffered 2-head groups)
  + 2 (O^T accumulators, head-pair passes) = 8 banks.
"""

import sys

sys.path.insert(0, "/opt/trn_rl_repo")

import ml_dtypes
import numpy as np

import concourse.bass as bass  # noqa: F401  (bass must import before bacc)
import concourse.mybir as mybir
from concourse import bacc
from concourse.bass_utils import run_bass_kernel_spmd
from concourse.tile import TileContext

F32R = mybir.dt.float32r
F32 = mybir.dt.float32
BF16 = mybir.dt.bfloat16
EXP = mybir.ActivationFunctionType.Exp

D = 1024
DK = 64
NHL = 4  # query heads per core
NCORES = 8

LAST_EXEC_NS = None
_NC_CACHE = {}


def build_nc(S):
    NQB = S // 512  # q-blocks of 512
    NKT = S // 128  # k-tiles of 128
    NSB = S // 512  # seq blocks of 512 (projections)

    nc = bacc.Bacc()
    # x^T sb-blocked so one [128, 8, 512] stream DMA is a single contiguous
    # 8KB run per partition (128 descriptors)
    qT = nc.declare_dram_parameter("qT", [128, S // 512, 8, 512], BF16, isOutput=False)
    kT = nc.declare_dram_parameter("kT", [128, S // 512, 8, 512], BF16, isOutput=False)
    vT = nc.declare_dram_parameter("vT", [128, S // 512, 8, 512], BF16, isOutput=False)
    wq = nc.declare_dram_parameter("wq", [128, 8, 256], BF16, isOutput=False)
    wk = nc.declare_dram_parameter("wk", [128, 8, 64], BF16, isOutput=False)
    wv = nc.declare_dram_parameter("wv", [128, 8, 64], BF16, isOutput=False)
    wo = nc.declare_dram_parameter("wo", [128, 2, D], BF16, isOutput=False)
    # ALiBi ext rows, all values exactly representable in bf16:
    # kext rows [k_hi, k_lo, 1, 1], qext rows [slope, slope, -s*q_hi, -s*q_lo]
    qext = nc.declare_dram_parameter("qext", [4, NHL, S], BF16, isOutput=False)
    kext = nc.declare_dram_parameter("kext", [4, S], BF16, isOutput=False)
    dmask = nc.declare_dram_parameter("dmask", [128, 2, 128], F32, isOutput=False)
    vones = nc.declare_dram_parameter("vones", [128, NKT, 1], BF16, isOutput=False)
    ident = nc.declare_dram_parameter("ident", [64, 64], BF16, isOutput=False)
    yT = nc.declare_dram_parameter("yT", [D, S], BF16, isOutput=True)

    with TileContext(nc) as tc:
        with (
            tc.sbuf_pool(name="cst", bufs=1) as cp,
            tc.sbuf_pool(name="xin", bufs=9) as xp,
            tc.sbuf_pool(name="stg", bufs=3) as sp,
            tc.sbuf_pool(name="pbuf", bufs=4) as pbp,
            tc.sbuf_pool(name="rbuf", bufs=6) as rp,
            tc.sbuf_pool(name="yout", bufs=4) as yp_sb,
            tc.psum_pool(name="pj", bufs=2) as pj,    # 2 banks: proj + outproj
            tc.psum_pool(name="sc", bufs=2) as scp,   # 4 banks: scores (double buffered)
            tc.psum_pool(name="ob", bufs=2) as obp,   # 2 banks: O^T accum (head pair)
        ):
            # ---- resident weights / constants ----
            # The ACT engine queue carries NO DMAs (CoreSim charges a queue's
            # DMA time to its engine, and ACT exp is the co-bottleneck).
            # Pool queue: weights + consts in criticality order; SP: x stream.
            qt_all = cp.tile([68, NHL, S], BF16)
            kt_sb = cp.tile([68, S], BF16)
            vext = cp.tile([128, NKT, 65], BF16)
            wq_sb = cp.tile([128, 8, 256], BF16)
            nc.gpsimd.dma_start(out=wq_sb[:], in_=wq[:])
            wk_sb = cp.tile([128, 8, 64], BF16)
            nc.gpsimd.dma_start(out=wk_sb[:], in_=wk[:])
            wv_sb = cp.tile([128, 8, 64], BF16)
            nc.gpsimd.dma_start(out=wv_sb[:], in_=wv[:])
            nc.gpsimd.dma_start(out=kt_sb[64:68, :], in_=kext[:])
            # pass-0 heads (0, 2) first, then pass-1 heads (1, 3)
            for hl in (0, 2, 1, 3):
                nc.gpsimd.dma_start(out=qt_all[64:68, hl, :], in_=qext[:, hl, :])
            nc.gpsimd.dma_start(out=vext[:, :, 64:65], in_=vones[:])
            ident_sb = cp.tile([64, 64], BF16)
            nc.gpsimd.dma_start(out=ident_sb[:], in_=ident[:])
            dmask_sb = cp.tile([128, 2, 128], F32)
            nc.gpsimd.dma_start(out=dmask_sb[:], in_=dmask[:])
            wo_sb = cp.tile([128, 2, D], BF16)
            nc.gpsimd.dma_start(out=wo_sb[:], in_=wo[:])

            otf = [cp.tile([128, S], BF16, name=f"otf{ch}") for ch in range(2)]

            def load_x(sb, split=False):
                """Prefetch x^T for seq block sb: one batched bf16 DMA per
                input ([128, 8, 512] = all 8 contraction chunks), SP queue.
                split=True halves each DMA so the first chunks land sooner."""
                xs = {}
                for nm, src in (("q", qT), ("k", kT), ("v", vT)):
                    t = xp.tile([128, 8, 512], BF16, name=f"x{nm}{sb}", tag="x")
                    if split:
                        nc.sync.dma_start(out=t[:, 0:4, :], in_=src[:, sb, 0:4, :])
                        nc.sync.dma_start(out=t[:, 4:8, :], in_=src[:, sb, 4:8, :])
                    else:
                        nc.sync.dma_start(out=t[:], in_=src[:, sb, :, :])
                    xs[nm] = t
                return xs

            def proj_units(sb, xs):
                """Q/K/V projections for seq block sb as schedulable units."""
                cols = slice(512 * sb, 512 * (sb + 1))
                xq = [xs["q"][:, ci, :] for ci in range(8)]
                xk = [xs["k"][:, ci, :] for ci in range(8)]
                xv = [xs["v"][:, ci, :] for ci in range(8)]
                units = []

                def q_unit(mt):
                    def run():
                        ps = pj.tile([128, 512], F32, name=f"qp{sb}_{mt}", tag="pj")
                        for ci in range(8):
                            nc.tensor.matmul(
                                ps[:],
                                lhsT=wq_sb[:, ci, 128 * mt : 128 * (mt + 1)],
                                rhs=xq[ci],
                                start=(ci == 0),
                                stop=(ci == 7),
                            )
                        heven, hodd = 2 * mt, 2 * mt + 1
                        # odd-head path (copy + shift DMA) is the longer leg
                        # of the exp dependency — emit it first
                        stg = sp.tile([128, 512], BF16, name=f"qs{sb}_{mt}", tag="st")
                        nc.vector.tensor_copy(stg[64:128, :], ps[64:128, :])
                        nc.gpsimd.dma_start(
                            out=qt_all[0:64, hodd, cols], in_=stg[64:128, :]
                        )
                        nc.vector.tensor_copy(qt_all[0:64, heven, cols], ps[0:64, :])

                    return run

                def k_unit():
                    ps = pj.tile([64, 512], F32, name=f"kp{sb}", tag="pj")
                    for ci in range(8):
                        nc.tensor.matmul(
                            ps[:],
                            lhsT=wk_sb[:, ci, :],
                            rhs=xk[ci],
                            start=(ci == 0),
                            stop=(ci == 7),
                        )
                    nc.vector.tensor_copy(kt_sb[0:64, cols], ps[:])

                vts = sp.tile([64, 512], BF16, name=f"vts{sb}", tag="vt")

                def v_unit():
                    ps = pj.tile([64, 512], F32, name=f"vtp{sb}", tag="pj")
                    for ci in range(8):
                        nc.tensor.matmul(
                            ps[:],
                            lhsT=wv_sb[:, ci, :],
                            rhs=xv[ci],
                            start=(ci == 0),
                            stop=(ci == 7),
                        )
                    nc.vector.tensor_copy(vts[:], ps[:])

                def t_unit(sub):
                    def run():
                        st_ = 4 * sb + sub
                        tp = pj.tile([128, 64], BF16, name=f"vtr{st_}", tag="pj")
                        nc.tensor.transpose(
                            tp[:], vts[:, 128 * sub : 128 * (sub + 1)], ident_sb[:]
                        )
                        nc.vector.tensor_copy(vext[:, st_, 0:64], tp[:])

                    return run

                units = [q_unit(0), q_unit(1), k_unit, v_unit]
                units += [t_unit(sub) for sub in range(4)]
                return units

            def attn_block(qb, pending=None):
                """Attention for q-block qb. Head passes are (0,2) then (1,3):
                pass 0 needs no qt partition-shift DMAs (even heads), giving
                the pass-1 shifts the whole pass-0 duration to land. One
                pending unit (outproj/proj) is emitted per ki iteration so PE
                interleaves them without starving the exp stream."""
                pending = list(pending or [])
                nk = 4 * qb + 4
                dst_cols = slice(512 * qb, 512 * (qb + 1))
                for pr in range(2):
                    heads = (pr, pr + 2)  # hh slot -> local head
                    ots = [
                        obp.tile([128, 512], F32, name=f"ot{qb}_{pr}_{hh}", tag="ot")
                        for hh in range(2)
                    ]
                    for ki in range(nk):
                        k0 = 128 * ki
                        diag = ki >= 4 * qb
                        if diag:
                            qs, w = k0, 512 - (k0 - 512 * qb)
                        else:
                            qs, w = 512 * qb, 512
                        qoff = qs - 512 * qb
                        scat = scp.tile(
                            [128, 2, 512], F32, name=f"s{qb}_{pr}_{ki}", tag="s"
                        )
                        for hh in range(2):
                            nc.tensor.matmul(
                                scat[:, hh, 0:w],
                                lhsT=kt_sb[:, k0 : k0 + 128],
                                rhs=qt_all[:, heads[hh], qs : qs + w],
                                start=True,
                                stop=True,
                            )
                        if diag:
                            nc.vector.tensor_add(
                                scat[:, :, 0:128], scat[:, :, 0:128], dmask_sb[:]
                            )
                        p = pbp.tile(
                            [128, 2, 512], BF16, name=f"p{qb}_{pr}_{ki}", tag="p"
                        )
                        nc.scalar.activation(p[:, :, 0:w], scat[:, :, 0:w], EXP)
                        for hh in range(2):
                            nc.tensor.matmul(
                                ots[hh][0:65, qoff : qoff + w],
                                lhsT=vext[:, ki, :],
                                rhs=p[:, hh, 0:w],
                                start=(ki == 0),
                                stop=(ki == nk - 1),
                            )
                        if pending and ki % 2 == 1:
                            pending.pop(0)()
                    # normalize the pair straight from PSUM: reciprocal of the
                    # denominator row, gpsimd partition-broadcast, multiply
                    for hh in range(2):
                        h = heads[hh]
                        rc = rp.tile([1, 512], F32, name=f"rc{qb}_{h}", tag="rc")
                        nc.vector.reciprocal(rc[:], ots[hh][64:65, :])
                        rb = rp.tile([64, 512], F32, name=f"rb{qb}_{h}", tag="rb")
                        nc.gpsimd.partition_broadcast(rb[:], rc[:])
                        ch, half = divmod(h, 2)
                        if half == 0:
                            nc.vector.tensor_mul(
                                otf[ch][0:64, dst_cols], ots[hh][0:64, :], rb[:]
                            )
                        else:
                            s2 = rp.tile([64, 512], BF16, name=f"os{qb}_{h}", tag="os")
                            nc.vector.tensor_mul(s2[:], ots[hh][0:64, :], rb[:])
                            nc.gpsimd.dma_start(
                                out=otf[ch][64:128, dst_cols], in_=s2[:]
                            )
                for u in pending:
                    u()

            def outproj_units(sb, act_copy=False):
                """Y^T columns for seq block sb as 8 schedulable units."""
                cols = slice(512 * sb, 512 * (sb + 1))

                def unit(yt):
                    def run():
                        ps = pj.tile([128, 512], F32, name=f"y{yt}_{sb}", tag="pj")
                        for ch in range(2):
                            nc.tensor.matmul(
                                ps[:],
                                lhsT=wo_sb[:, ch, 128 * yt : 128 * (yt + 1)],
                                rhs=otf[ch][:, cols],
                                start=(ch == 0),
                                stop=(ch == 1),
                            )
                        yo = yp_sb.tile(
                            [128, 512], BF16, name=f"yo{yt}_{sb}", tag="yo"
                        )
                        if act_copy:
                            nc.scalar.copy(yo[:], ps[:])
                        else:
                            nc.vector.tensor_copy(yo[:], ps[:])
                        nc.sync.dma_start(
                            out=yT[128 * yt : 128 * (yt + 1), cols], in_=yo[:]
                        )

                    return run

                return [unit(yt) for yt in range(8)]

            # interleaved schedule: proj(0) runs up front; during attn(qb) the
            # pending proj(qb+1) and outproj(qb-1) units are drip-fed into the
            # PE stream (proj first: it feeds the next attention block).
            # x is prefetched 3 seq blocks ahead on the dedicated SP queue.
            def interleave(a, b):
                out = []
                for i in range(max(len(a), len(b))):
                    if i < len(a):
                        out.append(a[i])
                    if i < len(b):
                        out.append(b[i])
                return out

            xss = [load_x(sb, split=(sb == 0)) for sb in range(min(3, NSB))]
            for u in proj_units(0, xss[0]):
                u()
            pending_op = []
            for qb in range(NQB):
                if qb + 3 < NSB:
                    xss.append(load_x(qb + 3))
                pu = proj_units(qb + 1, xss[qb + 1]) if qb + 1 < NSB else []
                attn_block(qb, interleave(pu, pending_op))
                pending_op = outproj_units(qb, act_copy=(qb == NQB - 1))
            for u in pending_op:
                u()

    nc.compile()
    return nc


def _get_nc(S):
    if S not in _NC_CACHE:
        _NC_CACHE[S] = build_nc(S)
    return _NC_CACHE[S]


def kernel(**inputs):
    global LAST_EXEC_NS
    f = np.float32
    query = np.asarray(inputs["query"], f)
    key = np.asarray(inputs["key"], f)
    value = np.asarray(inputs["value"], f)
    B, S, _ = query.shape

    scale = f(1.0 / np.sqrt(DK))
    Wq = np.asarray(inputs["Wq"], f) + np.asarray(inputs["Aq"], f) @ np.asarray(inputs["Bq"], f)
    Wq = Wq * scale
    Wk = np.asarray(inputs["Wk"], f) + np.asarray(inputs["Ak"], f) @ np.asarray(inputs["Bk"], f)
    Wv = np.asarray(inputs["Wv"], f) + np.asarray(inputs["Av"], f) @ np.asarray(inputs["Bv"], f)
    Wo = np.asarray(inputs["Wo"], f) + np.asarray(inputs["Ao"], f) @ np.asarray(inputs["Bo"], f)

    k_idx = np.arange(S, dtype=f)
    k_cent = k_idx - f(S // 2)
    # hi/lo split so every ext-row value is EXACT in bf16:
    # k_hi multiples of 16 (<=1024, 7 significand bits), k_lo in [-8, 8].
    k_hi = np.round(k_cent / 16) * 16
    k_lo = k_cent - k_hi
    kext = np.stack([k_hi, k_lo, np.ones(S, f), np.ones(S, f)])  # [4, S]
    # additive causal mask for the 128x128 diagonal block: 0 keep, -1e30 drop
    # (replicated for the 2-head score groups: [128, 2, 128])
    dm1 = np.where(np.triu(np.ones((128, 128), bool)), f(0), f(-1e30))
    dmask = np.ascontiguousarray(np.stack([dm1, dm1], axis=1))
    vones = np.ones((128, S // 128, 1), ml_dtypes.bfloat16)
    ident = np.eye(64, dtype=ml_dtypes.bfloat16)

    bf = ml_dtypes.bfloat16
    # x^T sb-blocked: xp[p, sb, ci, col] = x[512*sb + col, 128*ci + p]
    def pack_x(x):  # x: [S, D]
        t = x.reshape(S // 512, 512, 8, 128).transpose(3, 0, 2, 1)
        return np.ascontiguousarray(t).astype(bf)

    xpk = {b: (pack_x(query[b]), pack_x(key[b]), pack_x(value[b])) for b in range(B)}

    in_maps = []
    for c in range(NCORES):
        b, g = divmod(c, 4)
        qe = np.empty((4, NHL, S), f)
        for hl in range(NHL):
            slope = f(2.0 ** (-(4 * g + hl + 1)))
            qe[0, hl] = slope
            qe[1, hl] = slope
            qe[2, hl] = -slope * k_hi  # power-of-2 x multiple-of-16: exact
            qe[3, hl] = -slope * k_lo
        in_maps.append(
            {
                "qT": xpk[b][0],
                "kT": xpk[b][1],
                "vT": xpk[b][2],
                "wq": np.ascontiguousarray(
                    Wq[:, 256 * g : 256 * (g + 1)].reshape(8, 128, 256).transpose(1, 0, 2)
                ).astype(bf),
                "wk": np.ascontiguousarray(
                    Wk[:, 64 * g : 64 * (g + 1)].reshape(8, 128, 64).transpose(1, 0, 2)
                ).astype(bf),
                "wv": np.ascontiguousarray(
                    Wv[:, 64 * g : 64 * (g + 1)].reshape(8, 128, 64).transpose(1, 0, 2)
                ).astype(bf),
                "wo": np.ascontiguousarray(
                    Wo[256 * g : 256 * (g + 1), :].reshape(2, 128, D).transpose(1, 0, 2)
                ).astype(bf),
                "qext": qe.astype(bf),
                "kext": kext.astype(bf),
                "dmask": dmask,
                "vones": vones,
                "ident": ident,
            }
        )

    nc = _get_nc(S)
    res = run_bass_kernel_spmd(nc, in_maps, list(range(NCORES)))
    LAST_EXEC_NS = res.exec_time_ns

    out = np.empty((B, S, D), f)
    for b in range(B):
        acc = res.results[4 * b + 0]["yT"].astype(f)
        for g in range(1, 4):
            acc = acc + res.results[4 * b + g]["yT"].astype(f)
        out[b] = acc.T
    return out



# revision 68
# speedup vs baseline: 1.1059x; 1.1059x over previous
"""Trainium2 Bass kernel for GQA MultiHeadAttention + LoRA + ALiBi + causal mask.

Problem (hardcoded): B=2, S=2048, D=1024, H=16 q-heads, KVH=4 kv-heads, DK=64,
LoRA rank 16, ALiBi alpha 1.0, causal, softmax, eval mode.

Sharding over 8 cores: core c = 4*b + g handles batch b and kv-group g
(query heads 4g..4g+3, kv head g).

Host precompute:
  - LoRA folded into weights:  W_eff = W + A@B (SCALING=1).
  - Score scale (1/sqrt(64)) folded into W_eff_q.
  - Inputs passed pre-transposed (xT = x.T) because the PE contracts over the
    partition dim.
  - ALiBi folded into the score matmul as 2 extra contraction rows:
      kext = [k_idx; 1],  qext_h = [slope_h; -slope_h * q_idx]
    so S'[k,q] = scale*q.k + slope_h*(k - q) comes out of one matmul (K=66).

Device dataflow (per core, all matmul operands float32r = full-rate fp32):
  1. Projections into transposed layouts: Q^T (per head, 66 rows with ext),
     K^T (66 rows with ext), V natural [k,64] + ones column (row sums ride
     along the attention matmul: O^T_ext row 64 = softmax denominator).
  2. Per q-block of 512, per k-tile of 128: S^T = K^T_ext.T @ Q^T_ext on PE
     (contraction 66, [k, q] layout), exp on ACT over 2-head groups,
     causal diagonal handled by narrowing the q-span + one additive
     [128,128] mask (0 / -1e30) before exp, O^T accumulated in PSUM.
  3. Normalize: reciprocal of denominator row (DVE), broadcast across
     partitions (GpSimd), multiply (DVE).
  4. Output projection computed transposed (Y^T), its 8 column-tile units
     drip-fed into the next q-block's PE stream; host sums the 4 partial
     Y^T per batch and transposes.

  Schedule: proj(sb) / attention(sb-1) / outproj(sb-1) software-pipelined;
  PSUM budget 2 (proj+outproj) + 4 (scores, double-buffered 2-head groups)
  + 2 (O^T accumulators, head-pair passes) = 8 banks.
"""

import sys

sys.path.insert(0, "/opt/trn_rl_repo")

import ml_dtypes
import numpy as np

import concourse.bass as bass  # noqa: F401  (bass must import before bacc)
import concourse.mybir as mybir
from concourse import bacc
from concourse.bass_utils import run_bass_kernel_spmd
from concourse.tile import TileContext

F32R = mybir.dt.float32r
F32 = mybir.dt.float32
BF16 = mybir.dt.bfloat16
E4 = mybir.dt.float8e4
DR = mybir.MatmulPerfMode.DoubleRow
EXP = mybir.ActivationFunctionType.Exp

D = 1024
DK = 64
NHL = 4  # query heads per core
NCORES = 8

LAST_EXEC_NS = None
_NC_CACHE = {}


def build_nc(S):
    NQB = S // 512  # q-blocks of 512
    NKT = S // 128  # k-tiles of 128
    NSB = S // 512  # seq blocks of 512 (projections)

    nc = bacc.Bacc()
    # x^T sb-blocked so one [128, 8, 512] stream DMA is a single contiguous
    # 8KB run per partition (128 descriptors)
    qT = nc.declare_dram_parameter("qT", [128, S // 512, 8, 512], BF16, isOutput=False)
    kT = nc.declare_dram_parameter("kT", [128, S // 512, 8, 512], BF16, isOutput=False)
    vT = nc.declare_dram_parameter("vT", [128, S // 512, 8, 512], BF16, isOutput=False)
    wq = nc.declare_dram_parameter("wq", [128, 8, 256], BF16, isOutput=False)
    wk = nc.declare_dram_parameter("wk", [128, 8, 64], BF16, isOutput=False)
    wv = nc.declare_dram_parameter("wv", [128, 8, 64], BF16, isOutput=False)
    wo = nc.declare_dram_parameter("wo", [128, 2, D], BF16, isOutput=False)
    # ALiBi ext rows, all values exactly representable in bf16:
    # kext rows [k_hi, k_lo, 1, 1], qext rows [slope, slope, -s*q_hi, -s*q_lo]
    qext = nc.declare_dram_parameter("qext", [4, NHL, S], BF16, isOutput=False)
    kext = nc.declare_dram_parameter("kext", [4, S], BF16, isOutput=False)
    dmask = nc.declare_dram_parameter("dmask", [128, 2, 128], F32, isOutput=False)
    vones = nc.declare_dram_parameter("vones", [128, NKT, 1], BF16, isOutput=False)
    ident = nc.declare_dram_parameter("ident", [64, 64], BF16, isOutput=False)
    yT = nc.declare_dram_parameter("yT", [D, S], BF16, isOutput=True)

    with TileContext(nc) as tc:
        with (
            tc.sbuf_pool(name="cst", bufs=1) as cp,
            tc.sbuf_pool(name="xin", bufs=9) as xp,
            tc.sbuf_pool(name="stg", bufs=3) as sp,
            tc.sbuf_pool(name="pbuf", bufs=4) as pbp,
            tc.sbuf_pool(name="rbuf", bufs=6) as rp,
            tc.sbuf_pool(name="yout", bufs=4) as yp_sb,
            tc.psum_pool(name="pj", bufs=2) as pj,    # 2 banks: proj + outproj
            tc.psum_pool(name="sc", bufs=2) as scp,   # 4 banks: scores (double buffered)
            tc.psum_pool(name="ob", bufs=2) as obp,   # 2 banks: O^T accum (head pair)
        ):
            # ---- resident weights / constants ----
            # The ACT engine queue carries NO DMAs (CoreSim charges a queue's
            # DMA time to its engine, and ACT exp is the co-bottleneck).
            # Pool queue: weights + consts in criticality order; SP: x stream.
            qt_all = cp.tile([68, NHL, S], BF16)
            kt_sb = cp.tile([68, S], BF16)
            vext = cp.tile([128, NKT, 65], BF16)
            wq_sb = cp.tile([128, 8, 256], BF16)
            nc.gpsimd.dma_start(out=wq_sb[:], in_=wq[:])
            wk_sb = cp.tile([128, 8, 64], BF16)
            nc.gpsimd.dma_start(out=wk_sb[:], in_=wk[:])
            wv_sb = cp.tile([128, 8, 64], BF16)
            nc.gpsimd.dma_start(out=wv_sb[:], in_=wv[:])
            nc.gpsimd.dma_start(out=kt_sb[64:68, :], in_=kext[:])
            # pass-0 heads (0, 2) first, then pass-1 heads (1, 3)
            for hl in (0, 2, 1, 3):
                nc.gpsimd.dma_start(out=qt_all[64:68, hl, :], in_=qext[:, hl, :])
            nc.gpsimd.dma_start(out=vext[:, :, 64:65], in_=vones[:])
            ident_sb = cp.tile([64, 64], BF16)
            nc.gpsimd.dma_start(out=ident_sb[:], in_=ident[:])
            dmask_sb = cp.tile([128, 2, 128], F32)
            nc.gpsimd.dma_start(out=dmask_sb[:], in_=dmask[:])
            wo_sb = cp.tile([128, 2, D], BF16)
            nc.gpsimd.dma_start(out=wo_sb[:], in_=wo[:])

            otf = [cp.tile([128, S], BF16, name=f"otf{ch}") for ch in range(2)]

            def load_x(sb, split=False):
                """Prefetch x^T for seq block sb: one batched bf16 DMA per
                input ([128, 8, 512] = all 8 contraction chunks), SP queue.
                split=True halves each DMA so the first chunks land sooner."""
                xs = {}
                for nm, src in (("q", qT), ("k", kT), ("v", vT)):
                    t = xp.tile([128, 8, 512], BF16, name=f"x{nm}{sb}", tag="x")
                    if split:
                        npc = 4 if nm == "q" else 2
                        for pc in range(npc):
                            w8 = 8 // npc
                            nc.sync.dma_start(
                                out=t[:, w8 * pc : w8 * (pc + 1), :],
                                in_=src[:, sb, w8 * pc : w8 * (pc + 1), :],
                            )
                    else:
                        nc.sync.dma_start(out=t[:], in_=src[:, sb, :, :])
                    xs[nm] = t
                return xs

            def proj_units(sb, xs):
                """Q/K/V projections for seq block sb as schedulable units."""
                cols = slice(512 * sb, 512 * (sb + 1))
                xq = [xs["q"][:, ci, :] for ci in range(8)]
                xk = [xs["k"][:, ci, :] for ci in range(8)]
                xv = [xs["v"][:, ci, :] for ci in range(8)]
                units = []

                def q_unit(mt):
                    def run():
                        ps = pj.tile([128, 512], F32, name=f"qp{sb}_{mt}", tag="pj")
                        for ci in range(8):
                            nc.tensor.matmul(
                                ps[:],
                                lhsT=wq_sb[:, ci, 128 * mt : 128 * (mt + 1)],
                                rhs=xq[ci],
                                start=(ci == 0),
                                stop=(ci == 7),
                            )
                        heven, hodd = 2 * mt, 2 * mt + 1
                        # odd-head path (copy + shift DMA) is the longer leg
                        # of the exp dependency — emit it first
                        stg = sp.tile([128, 512], BF16, name=f"qs{sb}_{mt}", tag="st")
                        nc.vector.tensor_copy(stg[64:128, :], ps[64:128, :])
                        nc.gpsimd.dma_start(
                            out=qt_all[0:64, hodd, cols], in_=stg[64:128, :]
                        )
                        nc.vector.tensor_copy(qt_all[0:64, heven, cols], ps[0:64, :])

                    return run

                def k_unit():
                    ps = pj.tile([64, 512], F32, name=f"kp{sb}", tag="pj")
                    for ci in range(8):
                        nc.tensor.matmul(
                            ps[:],
                            lhsT=wk_sb[:, ci, :],
                            rhs=xk[ci],
                            start=(ci == 0),
                            stop=(ci == 7),
                        )
                    nc.vector.tensor_copy(kt_sb[0:64, cols], ps[:])

                vts = sp.tile([64, 512], BF16, name=f"vts{sb}", tag="vt")

                def v_unit():
                    ps = pj.tile([64, 512], F32, name=f"vtp{sb}", tag="pj")
                    for ci in range(8):
                        nc.tensor.matmul(
                            ps[:],
                            lhsT=wv_sb[:, ci, :],
                            rhs=xv[ci],
                            start=(ci == 0),
                            stop=(ci == 7),
                        )
                    nc.vector.tensor_copy(vts[:], ps[:])

                def t_unit(sub):
                    def run():
                        st_ = 4 * sb + sub
                        tp = pj.tile([128, 64], BF16, name=f"vtr{st_}", tag="pj")
                        nc.tensor.transpose(
                            tp[:], vts[:, 128 * sub : 128 * (sub + 1)], ident_sb[:]
                        )
                        nc.vector.tensor_copy(vext[:, st_, 0:64], tp[:])

                    return run

                units = [q_unit(0), q_unit(1), k_unit, v_unit]
                units += [t_unit(sub) for sub in range(4)]
                return units

            def attn_block(qb, pending=None, last=False):
                """Attention for q-block qb. Head passes are (0,2) then (1,3):
                pass 0 needs no qt partition-shift DMAs (even heads), giving
                the pass-1 shifts the whole pass-0 duration to land. (For the
                final block the order flips so the tail normalize is the
                cheap, shift-free even pass.) One pending unit (outproj/proj)
                is emitted per ki iteration so PE interleaves them without
                starving the exp stream."""
                pending = list(pending or [])
                nk = 4 * qb + 4
                dst_cols = slice(512 * qb, 512 * (qb + 1))
                for pr in (1, 0) if last else (0, 1):
                    heads = (pr, pr + 2)  # hh slot -> local head
                    ots = [
                        obp.tile([128, 512], F32, name=f"ot{qb}_{pr}_{hh}", tag="ot")
                        for hh in range(2)
                    ]
                    for ki in range(nk):
                        k0 = 128 * ki
                        diag = ki >= 4 * qb
                        if diag:
                            qs, w = k0, 512 - (k0 - 512 * qb)
                        else:
                            qs, w = 512 * qb, 512
                        qoff = qs - 512 * qb
                        scat = scp.tile(
                            [128, 2, 512], F32, name=f"s{qb}_{pr}_{ki}", tag="s"
                        )
                        for hh in range(2):
                            nc.tensor.matmul(
                                scat[:, hh, 0:w],
                                lhsT=kt_sb[:, k0 : k0 + 128],
                                rhs=qt_all[:, heads[hh], qs : qs + w],
                                start=True,
                                stop=True,
                            )
                        if diag:
                            nc.vector.tensor_add(
                                scat[:, :, 0:128], scat[:, :, 0:128], dmask_sb[:]
                            )
                        p = pbp.tile(
                            [128, 2, 512], BF16, name=f"p{qb}_{pr}_{ki}", tag="p"
                        )
                        nc.scalar.activation(p[:, :, 0:w], scat[:, :, 0:w], EXP)
                        for hh in range(2):
                            nc.tensor.matmul(
                                ots[hh][0:65, qoff : qoff + w],
                                lhsT=vext[:, ki, :],
                                rhs=p[:, hh, 0:w],
                                start=(ki == 0),
                                stop=(ki == nk - 1),
                            )
                        if pending and ki % 2 == 1:
                            pending.pop(0)()
                    # normalize the pair straight from PSUM: reciprocal of the
                    # denominator row, gpsimd partition-broadcast, multiply
                    for hh in range(2):
                        h = heads[hh]
                        rc = rp.tile([1, 512], F32, name=f"rc{qb}_{h}", tag="rc")
                        nc.vector.reciprocal(rc[:], ots[hh][64:65, :])
                        rb = rp.tile([64, 512], F32, name=f"rb{qb}_{h}", tag="rb")
                        nc.gpsimd.partition_broadcast(rb[:], rc[:])
                        ch, half = divmod(h, 2)
                        if half == 0:
                            nc.vector.tensor_mul(
                                otf[ch][0:64, dst_cols], ots[hh][0:64, :], rb[:]
                            )
                        else:
                            s2 = rp.tile([64, 512], BF16, name=f"os{qb}_{h}", tag="os")
                            nc.vector.tensor_mul(s2[:], ots[hh][0:64, :], rb[:])
                            nc.gpsimd.dma_start(
                                out=otf[ch][64:128, dst_cols], in_=s2[:]
                            )
                for u in pending:
                    u()

            def outproj_units(sb, act_copy=False):
                """Y^T columns for seq block sb as 8 schedulable units."""
                cols = slice(512 * sb, 512 * (sb + 1))

                def unit(yt):
                    def run():
                        ps = pj.tile([128, 512], F32, name=f"y{yt}_{sb}", tag="pj")
                        for ch in range(2):
                            nc.tensor.matmul(
                                ps[:],
                                lhsT=wo_sb[:, ch, 128 * yt : 128 * (yt + 1)],
                                rhs=otf[ch][:, cols],
                                start=(ch == 0),
                                stop=(ch == 1),
                            )
                        yo = yp_sb.tile(
                            [128, 512], BF16, name=f"yo{yt}_{sb}", tag="yo"
                        )
                        if act_copy and yt % 2 == 0:
                            nc.scalar.copy(yo[:], ps[:])
                        else:
                            nc.vector.tensor_copy(yo[:], ps[:])
                        nc.sync.dma_start(
                            out=yT[128 * yt : 128 * (yt + 1), cols], in_=yo[:]
                        )

                    return run

                return [unit(yt) for yt in range(8)]

            # interleaved schedule: proj(0) runs up front; during attn(qb) the
            # pending proj(qb+1) and outproj(qb-1) units are drip-fed into the
            # PE stream (proj first: it feeds the next attention block).
            # x is prefetched 3 seq blocks ahead on the dedicated SP queue.
            def interleave(a, b):
                out = []
                for i in range(max(len(a), len(b))):
                    if i < len(a):
                        out.append(a[i])
                    if i < len(b):
                        out.append(b[i])
                return out

            xss = [load_x(sb, split=(sb == 0)) for sb in range(min(3, NSB))]
            for u in proj_units(0, xss[0]):
                u()
            pending_op = []
            for qb in range(NQB):
                if qb + 3 < NSB:
                    xss.append(load_x(qb + 3))
                pu = proj_units(qb + 1, xss[qb + 1]) if qb + 1 < NSB else []
                attn_block(qb, interleave(pu, pending_op), last=(qb == NQB - 1))
                pending_op = outproj_units(qb, act_copy=(qb == NQB - 1))
            for u in pending_op:
                u()

    nc.compile()
    return nc


def _get_nc(S):
    if S not in _NC_CACHE:
        _NC_CACHE[S] = build_nc(S)
    return _NC_CACHE[S]


def kernel(**inputs):
    global LAST_EXEC_NS
    f = np.float32
    query = np.asarray(inputs["query"], f)
    key = np.asarray(inputs["key"], f)
    value = np.asarray(inputs["value"], f)
    B, S, _ = query.shape

    scale = f(1.0 / np.sqrt(DK))
    Wq = np.asarray(inputs["Wq"], f) + np.asarray(inputs["Aq"], f) @ np.asarray(inputs["Bq"], f)
    Wq = Wq * scale
    Wk = np.asarray(inputs["Wk"], f) + np.asarray(inputs["Ak"], f) @ np.asarray(inputs["Bk"], f)
    Wv = np.asarray(inputs["Wv"], f) + np.asarray(inputs["Av"], f) @ np.asarray(inputs["Bv"], f)
    Wo = np.asarray(inputs["Wo"], f) + np.asarray(inputs["Ao"], f) @ np.asarray(inputs["Bo"], f)

    k_idx = np.arange(S, dtype=f)
    k_cent = k_idx - f(S // 2)
    # hi/lo split so every ext-row value is EXACT in bf16:
    # k_hi multiples of 16 (<=1024, 7 significand bits), k_lo in [-8, 8].
    k_hi = np.round(k_cent / 16) * 16
    k_lo = k_cent - k_hi
    kext = np.stack([k_hi, k_lo, np.ones(S, f), np.ones(S, f)])  # [4, S]
    # additive causal mask for the 128x128 diagonal block: 0 keep, -1e30 drop
    # (replicated for the 2-head score groups: [128, 2, 128])
    dm1 = np.where(np.triu(np.ones((128, 128), bool)), f(0), f(-1e30))
    dmask = np.ascontiguousarray(np.stack([dm1, dm1], axis=1))
    vones = np.ones((128, S // 128, 1), ml_dtypes.bfloat16)
    ident = np.eye(64, dtype=ml_dtypes.bfloat16)

    bf = ml_dtypes.bfloat16
    # x^T sb-blocked: xp[p, sb, ci, col] = x[512*sb + col, 128*ci + p]
    def pack_x(x):  # x: [S, D]
        t = x.reshape(S // 512, 512, 8, 128).transpose(3, 0, 2, 1)
        return np.ascontiguousarray(t).astype(bf)

    xpk = {b: (pack_x(query[b]), pack_x(key[b]), pack_x(value[b])) for b in range(B)}

    in_maps = []
    for c in range(NCORES):
        b, g = divmod(c, 4)
        qe = np.empty((4, NHL, S), f)
        for hl in range(NHL):
            slope = f(2.0 ** (-(4 * g + hl + 1)))
            qe[0, hl] = slope
            qe[1, hl] = slope
            qe[2, hl] = -slope * k_hi  # power-of-2 x multiple-of-16: exact
            qe[3, hl] = -slope * k_lo
        in_maps.append(
            {
                "qT": xpk[b][0],
                "kT": xpk[b][1],
                "vT": xpk[b][2],
                "wq": np.ascontiguousarray(
                    Wq[:, 256 * g : 256 * (g + 1)].reshape(8, 128, 256).transpose(1, 0, 2)
                ).astype(bf),
                "wk": np.ascontiguousarray(
                    Wk[:, 64 * g : 64 * (g + 1)].reshape(8, 128, 64).transpose(1, 0, 2)
                ).astype(bf),
                "wv": np.ascontiguousarray(
                    Wv[:, 64 * g : 64 * (g + 1)].reshape(8, 128, 64).transpose(1, 0, 2)
                ).astype(bf),
                "wo": np.ascontiguousarray(
                    Wo[256 * g : 256 * (g + 1), :].reshape(2, 128, D).transpose(1, 0, 2)
                ).astype(bf),
                "qext": qe.astype(bf),
                "kext": kext.astype(bf),
                "dmask": dmask,
                "vones": vones,
                "ident": ident,
            }
        )

    nc = _get_nc(S)
    res = run_bass_kernel_spmd(nc, in_maps, list(range(NCORES)))
    LAST_EXEC_NS = res.exec_time_ns

    out = np.empty((B, S, D), f)
    for b in range(B):
        acc = res.results[4 * b + 0]["yT"].astype(f)
        for g in range(1, 4):
            acc = acc + res.results[4 * b + g]["yT"].astype(f)
        out[b] = acc.T
    return out

